# revision 19
# baseline (speedup 1.0000x reference)
"""VisionZip (CLIPVisionTower EXP) Trainium2 Bass kernel.

Full-input contract: kernel(**inputs) takes the unsharded inputs and
returns (out [16,65,1024] f32, all_indices [16,55] i32), matching the
reference. Pure data parallel over 8 NeuronCores, 2 samples per core;
all compute on-device.

Self-contained: shapes hardcoded for B=16, H=16, L=577, C=1024, Ck=64,
dominant=54, contextual=10, 8 cores.

Structure per core (b = 2 samples):
  scores:  s_attn (CLS-attention head-sum, PE) +
           H_feat (streaming softmax entropy over Ck) +
           I_mut (1 - H_sim of the token-token cosine softmax).
           H_sim row sums use the symmetry of e2 = exp(10*sim-10):
           row-sum(e2*sim) and row-sum(e2) come from PE matmuls
           W' = e2 @ [z | 1] instead of DVE reductions.
  top-54:  7 rounds of DVE max8/max_index/match_replace on [2,576].
  order:   prefix-scan of the selection mask -> scatter offsets; the
           ordered dominant gather is an indirect-DMA row scatter.
  merge:   argmax-assign of remaining tokens onto 10 target slots in
           token space; aggregation is a PE matmul with a masked
           one-hot; contextual = hidden[tgt] + aggregated/counts.
"""
import math
import os

import numpy as np

import concourse.bass as bass
import concourse.bacc as bacc
import concourse.tile as tile
from concourse import mybir
from concourse.masks import make_identity

F32 = mybir.dt.float32
F32R = mybir.dt.float32r
I32 = mybir.dt.int32
U32 = mybir.dt.uint32
OP = mybir.AluOpType
ACT = mybir.ActivationFunctionType

B, H, L, C, CK = 16, 16, 577, 1024, 64
N = L - 1                  # 576 scored tokens
DOM = 54
KCTX = 10
D1 = DOM + 1               # 55 dominant incl CLS
NF = L - D1                # 522 filtered tokens
STEP = NF // KCTX          # 52
OROW = D1 + KCTX + 1       # 66 output rows per sample (65 + trash)
NCORES = 8
BPC = B // NCORES

NEG = -1e9
SENT = -1e30
TRASH = 1 << 20
EPS = 1e-12

SROWS = [128, 128, 128, 128, 64]   # score-space chunks (tokens 1..576)
TROWS = [128, 128, 128, 128, 65]   # token-space chunks (0..576)


def build_program():
    f32r_sim = os.environ.get("VZ_F32R_SIM", "1") == "1"
    nc = bacc.Bacc("TRN2", target_bir_lowering=False, debug=False)

    attn = nc.dram_tensor("attn", [BPC, H, L, L], F32, kind="ExternalInput")
    hidden = nc.dram_tensor("hidden", [BPC * L, C], F32, kind="ExternalInput")
    metric = nc.dram_tensor("metric", [BPC * L, CK], F32, kind="ExternalInput")
    hsel_in = nc.dram_tensor("hsel", [2 * H, BPC], F32, kind="ExternalInput")
    w01_in = nc.dram_tensor("w01", [2, 1], F32, kind="ExternalInput")
    iotaf_in = nc.dram_tensor("iotaf", [2, 640], F32, kind="ExternalInput")
    iotai_in = nc.dram_tensor("iotai", [640], I32, kind="ExternalInput")
    rb_in = nc.dram_tensor("rb", [2, 4], F32, kind="ExternalInput")

    out = nc.dram_tensor("out", [BPC * OROW, C], F32, kind="ExternalOutput")
    all_idx = nc.dram_tensor("all_idx", [BPC, D1], I32, kind="ExternalOutput")

    scr_dom = nc.dram_tensor("scr_dom", [BPC * L], I32)
    scr_am = nc.dram_tensor("scr_am", [BPC * L], F32)
    scr_fo = nc.dram_tensor("scr_fo", [BPC * L], I32)
    filt_l = nc.dram_tensor("filt_l", [BPC * (NF + 1)], I32)

    LOG64 = float(np.float32(math.log(64.0)))
    LOGN = float(np.float32(math.log(float(N))))

    def rr(ap):
        return ap.bitcast(F32R) if f32r_sim else ap

    with tile.TileContext(nc) as tc:
        with (
            tc.tile_pool(name="cst", bufs=1) as cst,
            tc.tile_pool(name="big", bufs=1) as big,
            tc.tile_pool(name="wk", bufs=3) as wk,
            tc.tile_pool(name="row", bufs=1) as row,
            tc.tile_pool(name="wide", bufs=2) as wide,
            tc.tile_pool(name="outc", bufs=1) as outc,
            tc.tile_pool(name="sm", bufs=4) as sm,
            tc.tile_pool(name="psA", bufs=2, space="PSUM") as psA,   # 2-bank tiles
            tc.tile_pool(name="psB", bufs=2, space="PSUM") as psB,   # 1-bank tiles x2
        ):
            # ---------------- constants ----------------
            ident = cst.tile([128, 128], F32)
            make_identity(nc, ident[:])
            hsel = cst.tile([2 * H, BPC], F32)
            nc.sync.dma_start(hsel[:], hsel_in[:])
            w01 = cst.tile([2, 1], F32)
            nc.sync.dma_start(w01[:], w01_in[:])
            iotaf = cst.tile([2, 640], F32)
            nc.sync.dma_start(iotaf[:], iotaf_in[:])
            rb = cst.tile([2, 4], F32)      # cols: OROW*b-1, (NF+1)*b, L*b, 0
            nc.sync.dma_start(rb[:], rb_in[:])
            iotac = []
            for c in range(5):
                t = cst.tile([128, 1], I32, tag=f"iotac{c}")
                nc.sync.dma_start(t[:], iotai_in[128 * c:128 * (c + 1)].unsqueeze(1))
                iotac.append(t)
            ones2 = cst.tile([2, 1], F32)
            nc.vector.memset(ones2[:], 1.0)
            nbias10 = cst.tile([128, 1], F32)
            nc.vector.memset(nbias10[:], -10.0)
            onesr = cst.tile([2, L], F32)
            nc.vector.memset(onesr[:], 1.0)
            onesc = cst.tile([128, 1], F32)
            nc.vector.memset(onesc[:], 1.0)
            zcol = cst.tile([128, 1], F32)
            nc.vector.memset(zcol[:], 0.0)

            # ---------------- early bulk loads ----------------
            hs_t = {}
            for b in range(BPC):
                for c in range(5):
                    r = TROWS[c]
                    hs = big.tile([128, C], F32, tag=f"hs{b}_{c}")
                    nc.sync.dma_start(hs[:r, :], hidden[b * L + 128 * c:b * L + 128 * c + r, :])
                    hs_t[(b, c)] = hs
            at2 = big.tile([2 * H, N], F32)
            for b in range(BPC):
                nc.sync.dma_start(at2[H * b:H * (b + 1), :], attn[b, :, 0, 1:])

            # ================= score phase =================
            # metric tiles: both samples side by side [128, 2, 64]
            z_t = {}
            mnT = {}
            e2_t = {}
            for b in range(BPC):
                m = big.tile([CK, L], F32R if f32r_sim else F32, tag=f"mnT{b}")
                nc.vector.tensor_copy(out=m[:, 0:1], in_=zcol[:CK, :])
                mnT[b] = m
            # per-row stats, col layout c = 2*t + b (plus Hf/Im halves)
            zall = big.tile([128, 20], F32)     # cols 0-9: Zf(t,b); 10-19: Z2(t,b)
            uall = big.tile([128, 20], F32)     # cols 0-9: Uf(t,b); 10-19: U2(t,b)
            nc.gpsimd.memset(zall[:], 1.0)      # t=4 rows 64:128 stay unwritten
            nc.gpsimd.memset(uall[:], 1.0)

            for t in range(5):
                r = SROWS[t]
                x2 = wk.tile([128, 2, CK], F32, tag="x2")
                for b in range(BPC):
                    tok0 = 1 + 128 * t
                    nc.sync.dma_start(x2[:r, b, :], metric[b * L + tok0:b * L + tok0 + r, :])
                # H_feat pieces (no max-subtraction; |5x| < 30 is exp-safe)
                ef = wk.tile([128, 2, CK], F32, tag="ef")
                nc.scalar.activation(out=ef[:r], in_=x2[:r], func=ACT.Exp, scale=5.0)
                nc.vector.reduce_sum(out=zall[:r, t:t + 6:5], in_=ef[:r],
                                     axis=mybir.AxisListType.X)
                pf = wk.tile([128, 2, CK], F32, tag="pf")
                nc.vector.tensor_mul(out=pf[:r], in0=ef[:r], in1=x2[:r])
                nc.vector.reduce_sum(out=uall[:r, t:t + 6:5], in_=pf[:r],
                                     axis=mybir.AxisListType.X)
                # normalize rows -> z (paired), transpose into mnT per sample
                sq = wk.tile([128, 2, CK], F32, tag="sq")
                nc.scalar.activation(out=sq[:r], in_=x2[:r], func=ACT.Square)
                ss2 = sm.tile([128, 2], F32, tag="ss2")
                nc.vector.reduce_sum(out=ss2[:r, :], in_=sq[:r], axis=mybir.AxisListType.X)
                nc.scalar.activation(out=ss2[:r, :], in_=ss2[:r, :], func=ACT.Sqrt)
                nc.vector.tensor_scalar(out=ss2[:r, :], in0=ss2[:r, :], scalar1=EPS,
                                        scalar2=None, op0=OP.add)
                rn2 = sm.tile([128, 2], F32, tag="rn2")
                nc.vector.reciprocal(out=rn2[:r, :], in_=ss2[:r, :])
                z2t = big.tile([128, 2, CK + 1], F32, tag=f"z2t{t}")
                nc.vector.tensor_mul(out=z2t[:r, :, 0:CK], in0=x2[:r],
                                     in1=rn2[:r, :].unsqueeze(2).to_broadcast([r, 2, CK]))
                nc.vector.memset(z2t[:r, :, CK:CK + 1], 1.0)
                for b in range(BPC):
                    tok0 = 1 + 128 * t
                    pzt = psB.tile([CK, 128], F32, tag="tr", space="PSUM")
                    nc.tensor.transpose(out=pzt[:, :r], in_=z2t[:r, b, 0:CK],
                                        identity=ident[:r, :r])
                    nc.vector.tensor_copy(out=mnT[b][:, tok0:tok0 + r], in_=pzt[:, :r])
                z_t[t] = z2t

            # sim rows: psim = z @ z^T with -1e9 diag; e2 = exp(10s-10)
            for t in range(5):
                r = SROWS[t]
                for b in range(BPC):
                    tok0 = 1 + 128 * t
                    psim = psA.tile([128, N], F32, tag="psim", space="PSUM")
                    nc.tensor.matmul(psim[:r, 0:512], lhsT=mnT[b][:, tok0:tok0 + r],
                                     rhs=mnT[b][:, 1:513], start=True, stop=True)
                    nc.tensor.matmul(psim[:r, 512:N], lhsT=mnT[b][:, tok0:tok0 + r],
                                     rhs=mnT[b][:, 513:L], start=True, stop=True)
                    e2 = big.tile([128, N], F32, tag=f"e2_{b}_{t}")
                    nc.scalar.activation(out=e2[:r, :], in_=psim[:r, :], func=ACT.Exp,
                                         bias=nbias10[:r, :], scale=10.0)
                    nc.gpsimd.affine_select(out=e2[:r, :], in_=e2[:r, :],
                                            compare_op=OP.not_equal, fill=0.0,
                                            base=128 * t, channel_multiplier=1,
                                            pattern=[[-1, N]])
                    e2_t[(b, t)] = e2

            # W' = e2 @ [z|1]  -> row sums; U2 = z . W[:, :CK], Z2 = W[:, CK]
            for t in range(5):
                r = SROWS[t]
                for b in range(BPC):
                    pw = psB.tile([128, CK + 1], F32, tag="W", space="PSUM")
                    for j in range(5):
                        rj = SROWS[j]
                        nc.tensor.matmul(pw[:r, :], lhsT=e2_t[(b, j)][:rj, 128 * t:128 * t + r],
                                         rhs=z_t[j][:rj, b, :], start=(j == 0), stop=(j == 4))
                    uz = wk.tile([128, CK], F32, tag="uz")
                    nc.vector.tensor_mul(out=uz[:r, :], in0=z_t[t][:r, b, 0:CK],
                                         in1=pw[:r, 0:CK])
                    nc.vector.reduce_sum(out=uall[:r, 10 + 5 * b + t:11 + 5 * b + t],
                                         in_=uz[:r, :], axis=mybir.AxisListType.X)
                    nc.vector.tensor_copy(out=zall[:r, 10 + 5 * b + t:11 + 5 * b + t],
                                          in_=pw[:r, CK:CK + 1])

            # batched per-row entropy math on [128, 20]
            lnz = big.tile([128, 20], F32)
            nc.scalar.activation(out=lnz[:], in_=zall[:], func=ACT.Ln)
            rz = big.tile([128, 20], F32)
            nc.vector.reciprocal(out=rz[:], in_=zall[:])
            uoz = big.tile([128, 20], F32)
            nc.vector.tensor_mul(out=uoz[:], in0=uall[:], in1=rz[:])
            # Hf = lnZf/LOG64 - 5*(Uf/Zf)/LOG64          (cols 0-9)
            # Im = 10*(U2/Z2)/LOGN - lnZ2/LOGN + (1-10/LOGN)   (cols 10-19)
            hfim_c = big.tile([128, 20], F32)
            nc.vector.tensor_scalar(out=hfim_c[:, 0:10], in0=uoz[:, 0:10],
                                    scalar1=-5.0 / LOG64, scalar2=None, op0=OP.mult)
            t_hf = wk.tile([128, 10], F32, tag="t_hf")
            nc.vector.tensor_scalar(out=t_hf[:], in0=lnz[:, 0:10],
                                    scalar1=1.0 / LOG64, scalar2=None, op0=OP.mult)
            nc.vector.tensor_add(out=hfim_c[:, 0:10], in0=hfim_c[:, 0:10], in1=t_hf[:])
            nc.vector.tensor_scalar(out=hfim_c[:, 10:20], in0=uoz[:, 10:20],
                                    scalar1=10.0 / LOGN, scalar2=1.0 - 10.0 / LOGN,
                                    op0=OP.mult, op1=OP.add)
            t_im = wk.tile([128, 10], F32, tag="t_im")
            nc.vector.tensor_scalar(out=t_im[:], in0=lnz[:, 10:20],
                                    scalar1=-1.0 / LOGN, scalar2=None, op0=OP.mult)
            nc.vector.tensor_add(out=hfim_c[:, 10:20], in0=hfim_c[:, 10:20], in1=t_im[:])

            # transpose col-stats into per-sample rows [2,576]: Hf row0, Im row1
            hfim = {}
            for b in range(BPC):
                hf_tile = big.tile([2, N], F32, tag=f"hfim{b}", name=f"hfim{b}")
                hfim[b] = hf_tile
            pth = psB.tile([20, 128], F32, tag="W", space="PSUM")
            nc.tensor.transpose(out=pth[:], in_=hfim_c[:], identity=ident[:])
            sth = wk.tile([20, 128], F32, tag="sth")
            nc.vector.tensor_copy(out=sth[:], in_=pth[:])
            # psum rows: col index 2t+b (Hf), 10+2t+b (Im)
            for b in range(BPC):
                for comp in (0, 1):
                    base_r = comp * 10 + b * 5
                    nc.sync.dma_start(hfim[b][comp:comp + 1, 0:512],
                                      sth[base_r:base_r + 4, :])
                    nc.sync.dma_start(hfim[b][comp:comp + 1, 512:576],
                                      sth[base_r + 4:base_r + 5, 0:64])

            # s_attn rows via head-sum matmul
            sar = {}
            for b in range(BPC):
                psa = psA.tile([1, N], F32, tag="psim", space="PSUM")
                nc.tensor.matmul(psa[:, 0:512], lhsT=hsel[:, b:b + 1],
                                 rhs=at2[:, 0:512], start=True, stop=True)
                nc.tensor.matmul(psa[:, 512:N], lhsT=hsel[:, b:b + 1],
                                 rhs=at2[:, 512:N], start=True, stop=True)
                s = row.tile([1, N], F32, tag=f"sar{b}")
                nc.vector.tensor_copy(out=s[:], in_=psa[:])
                sar[b] = s

            # zscore (ddof=1): t <- (t - mean)/(std+EPS) * w
            def zscore(tl, p, wcol):
                s = sm.tile([p, 1], F32, tag="zs_s")
                nc.vector.reduce_sum(out=s[:], in_=tl[:], axis=mybir.AxisListType.X)
                nc.vector.tensor_scalar(out=s[:], in0=s[:], scalar1=1.0 / N,
                                        scalar2=None, op0=OP.mult)
                nc.vector.tensor_scalar(out=tl[:], in0=tl[:], scalar1=s[:],
                                        scalar2=None, op0=OP.subtract)
                sqj = wide.tile([p, N], F32, tag="sqj")
                ssq = sm.tile([p, 1], F32, tag="zs_ss")
                nc.scalar.activation(out=sqj[:], in_=tl[:], func=ACT.Square,
                                     accum_out=ssq[:])
                nc.vector.tensor_scalar(out=ssq[:], in0=ssq[:], scalar1=1.0 / (N - 1),
                                        scalar2=None, op0=OP.mult)
                nc.scalar.activation(out=ssq[:], in_=ssq[:], func=ACT.Sqrt)
                nc.vector.tensor_scalar(out=ssq[:], in0=ssq[:], scalar1=EPS,
                                        scalar2=None, op0=OP.add)
                rstd = sm.tile([p, 1], F32, tag="zs_rstd")
                nc.vector.reciprocal(out=rstd[:], in_=ssq[:])
                if wcol is not None:
                    nc.vector.tensor_mul(out=rstd[:], in0=rstd[:], in1=wcol[:])
                nc.vector.tensor_scalar(out=tl[:], in0=tl[:], scalar1=rstd[:],
                                        scalar2=None, op0=OP.mult)

            work = big.tile([2, N], F32)       # both samples' scores
            for b in range(BPC):
                zscore(sar[b], 1, None)
                zscore(hfim[b], 2, w01)
                ph = psA.tile([1, N], F32, tag="psim", space="PSUM")
                nc.tensor.matmul(ph[:, 0:512], lhsT=ones2[:], rhs=hfim[b][:, 0:512],
                                 start=True, stop=True)
                nc.tensor.matmul(ph[:, 512:N], lhsT=ones2[:], rhs=hfim[b][:, 512:N],
                                 start=True, stop=True)
                sc_b = row.tile([1, N], F32, tag=f"sc{b}")
                nc.vector.tensor_add(out=sc_b[:], in0=sar[b][:], in1=ph[:])
                nc.sync.dma_start(work[b:b + 1, :], sc_b[:])

            # ================= top-54 (joint, 7 rounds) =================
            topidx = big.tile([2, 56], U32)
            mx8 = sm.tile([2, 8], F32, tag="mx8")
            for rnd in range(7):
                nc.vector.max(out=mx8[:], in_=work[:])
                if rnd == 6:
                    nc.vector.memset(mx8[:, 6:8], 1e30)
                nc.vector.max_index(topidx[:, 8 * rnd:8 * rnd + 8], mx8[:], work[:])
                nc.vector.match_replace(out=work[:], in_to_replace=mx8[:],
                                        in_values=work[:], imm_value=SENT)

            aidx = row.tile([2, D1], F32, tag="aidx")
            nc.vector.memset(aidx[:, 0:1], 0.0)
            topf = row.tile([2, DOM], F32, tag="topf")
            nc.vector.tensor_copy(out=topf[:], in_=topidx[:, 0:DOM])
            nc.vector.tensor_scalar(out=aidx[:, 1:D1], in0=topf[:], scalar1=1.0,
                                    scalar2=None, op0=OP.add)
            aidx_i = row.tile([2, D1], I32, tag="aidx_i")
            nc.vector.tensor_copy(out=aidx_i[:], in_=aidx[:])
            nc.sync.dma_start(all_idx[:, :], aidx_i[:, :])

            # ============ mask / prefix / offsets (joint [2,577]) ============
            mask = row.tile([2, L], I32, tag="mask")
            nc.gpsimd.memset(mask[:, 0:1], 1)
            nc.gpsimd.tensor_scalar(out=mask[:, 1:L], in0=work[:], scalar1=-1e29,
                                    scalar2=None, op0=OP.is_lt)
            notm = row.tile([2, L], I32, tag="notm")
            nc.gpsimd.memset(notm[:, 0:1], 0)
            nc.gpsimd.tensor_scalar(out=notm[:, 1:L], in0=work[:], scalar1=-1e29,
                                    scalar2=None, op0=OP.is_ge)
            mask_f = row.tile([2, L], F32, tag="mask_f")
            nc.gpsimd.tensor_copy(out=mask_f[:], in_=mask[:])
            pd = row.tile([2, L], F32, tag="pd")
            nc.vector.tensor_tensor_scan(out=pd[:], data0=mask_f[:], data1=onesr[:],
                                         initial=0.0, op0=OP.add, op1=OP.mult)
            # dominant offsets: sel ? pd-1+OROW*b : TRASH
            domo = row.tile([2, L], F32, tag="domo")
            nc.gpsimd.memset(domo[:], float(TRASH))
            pdt = row.tile([2, L], F32, tag="pdt")
            nc.vector.tensor_scalar(out=pdt[:], in0=pd[:], scalar1=rb[:, 0:1],
                                    scalar2=None, op0=OP.add)
            nc.vector.copy_predicated(out=domo[:], mask=mask[:], data=pdt[:])
            domo_i = row.tile([2, L], I32, tag="domo_i")
            nc.gpsimd.tensor_copy(out=domo_i[:], in_=domo[:])
            nc.sync.dma_start(scr_dom[:], domo_i[:, :])
            # filtered offsets: !sel ? i-pd+(NF+1)*b : TRASH
            filo = row.tile([2, L], F32, tag="filo")
            nc.gpsimd.memset(filo[:], float(TRASH))
            fpos = row.tile([2, L], F32, tag="fpos")
            nc.vector.tensor_sub(out=fpos[:], in0=iotaf[:, 0:L], in1=pd[:])
            nc.vector.tensor_scalar(out=fpos[:], in0=fpos[:], scalar1=rb[:, 1:2],
                                    scalar2=None, op0=OP.add)
            nc.vector.copy_predicated(out=filo[:], mask=notm[:], data=fpos[:])
            filo_i = row.tile([2, L], I32, tag="filo_i")
            nc.gpsimd.tensor_copy(out=filo_i[:], in_=filo[:])
            nc.sync.dma_start(scr_fo[:], filo_i[:, :])
            notm_f = row.tile([2, L], F32, tag="notm_f")
            nc.gpsimd.tensor_copy(out=notm_f[:], in_=notm[:])
            nc.sync.dma_start(scr_am[:], notm_f[:, :])

            # filt token list scatter
            for b in range(BPC):
                for c in range(5):
                    r = TROWS[c]
                    oc = wk.tile([128, 1], I32, tag="oc")
                    nc.sync.dma_start(oc[:r, :], scr_fo[b * L + 128 * c:b * L + 128 * c + r].unsqueeze(1))
                    nc.gpsimd.indirect_dma_start(
                        out=filt_l[:].unsqueeze(1), in_=iotac[c][:r, :],
                        out_offset=bass.IndirectOffsetOnAxis(ap=oc[:r, 0:1], axis=0),
                        in_offset=None,
                        bounds_check=BPC * (NF + 1) - 1, oob_is_err=False)

            # ============ merge phase ============
            for b in range(BPC):
                # tgt token ids
                tgt = sm.tile([KCTX, 1], I32, tag="tgt")
                nc.sync.dma_start(
                    tgt[:], filt_l[b * (NF + 1):b * (NF + 1) + STEP * KCTX:STEP].unsqueeze(1))
                tgtf = sm.tile([KCTX, 1], F32, tag="tgtf")
                nc.vector.tensor_copy(out=tgtf[:], in_=tgt[:])
                if b > 0:
                    nc.vector.tensor_scalar(out=tgtf[:], in0=tgtf[:], scalar1=float(b * L),
                                            scalar2=None, op0=OP.add)
                tgtg = sm.tile([KCTX, 1], I32, tag="tgtg")
                nc.vector.tensor_copy(out=tgtg[:], in_=tgtf[:])
                zee = sm.tile([KCTX, 1], F32, tag="zee")
                nc.vector.memset(zee[:], 0.0)
                nc.gpsimd.indirect_dma_start(
                    out=scr_am[:].unsqueeze(1), in_=zee[:],
                    out_offset=bass.IndirectOffsetOnAxis(ap=tgtg[:, 0:1], axis=0),
                    in_offset=None)

                # tgt metric rows -> normalize -> [64,10]
                mt = wk.tile([KCTX, CK], F32, tag="mt")
                nc.gpsimd.indirect_dma_start(
                    out=mt[:], out_offset=None, in_=metric[:, :],
                    in_offset=bass.IndirectOffsetOnAxis(ap=tgtg[:, 0:1], axis=0))
                mtsq = wk.tile([KCTX, CK], F32, tag="mtsq")
                sst = sm.tile([KCTX, 1], F32, tag="sst")
                nc.scalar.activation(out=mtsq[:], in_=mt[:], func=ACT.Square,
                                     accum_out=sst[:])
                nc.scalar.activation(out=sst[:], in_=sst[:], func=ACT.Sqrt)
                nc.vector.tensor_scalar(out=sst[:], in0=sst[:], scalar1=EPS,
                                        scalar2=None, op0=OP.add)
                rnt = sm.tile([KCTX, 1], F32, tag="rnt")
                nc.vector.reciprocal(out=rnt[:], in_=sst[:])
                nc.vector.tensor_scalar(out=mt[:], in0=mt[:], scalar1=rnt[:],
                                        scalar2=None, op0=OP.mult)
                ptg = psB.tile([CK, KCTX], F32, tag="tr", space="PSUM")
                nc.tensor.transpose(out=ptg[:, :], in_=mt[:, :], identity=ident[:KCTX, :KCTX])
                tgtT = wk.tile([CK, KCTX], F32, tag="tgtT")
                nc.vector.tensor_copy(out=tgtT[:], in_=ptg[:])

                pcnt = psB.tile([1, KCTX], F32, tag="tr", space="PSUM")
                pagg = psA.tile([KCTX, C], F32, tag="psim", space="PSUM")
                for c in range(5):
                    r = TROWS[c]
                    hs = hs_t[(b, c)]
                    dc = wk.tile([128, 1], I32, tag="dc")
                    nc.sync.dma_start(dc[:r, :], scr_dom[b * L + 128 * c:b * L + 128 * c + r].unsqueeze(1))
                    nc.gpsimd.indirect_dma_start(
                        out=out[:, :], in_=hs[:r, :],
                        out_offset=bass.IndirectOffsetOnAxis(ap=dc[:r, 0:1], axis=0),
                        in_offset=None,
                        bounds_check=BPC * OROW - 1, oob_is_err=False)
                    amc = wk.tile([128, 1], F32, tag="amc")
                    nc.sync.dma_start(amc[:r, :], scr_am[b * L + 128 * c:b * L + 128 * c + r].unsqueeze(1))
                    ps2 = psB.tile([128, KCTX], F32, tag="W", space="PSUM")
                    nc.tensor.matmul(ps2[:r, :], lhsT=mnT[b][:, 128 * c:128 * c + r].bitcast(F32),
                                     rhs=tgtT[:], start=True, stop=True)
                    mx2 = sm.tile([128, 1], F32, tag="mx2")
                    nc.vector.reduce_max(out=mx2[:r, :], in_=ps2[:r, :],
                                         axis=mybir.AxisListType.X)
                    asn = wk.tile([128, KCTX], F32, tag="asn")
                    nc.vector.tensor_scalar(out=asn[:r, :], in0=ps2[:r, :],
                                            scalar1=mx2[:r, :], scalar2=amc[:r, :],
                                            op0=OP.is_ge, op1=OP.mult)
                    nc.tensor.matmul(pcnt[:], lhsT=onesc[:r, :], rhs=asn[:r, :].bitcast(F32),
                                     start=(c == 0), stop=(c == 4))
                    nc.tensor.matmul(pagg[:, 0:512], lhsT=asn[:r, :].bitcast(F32),
                                     rhs=hs[:r, 0:512], start=(c == 0), stop=(c == 4))
                    nc.tensor.matmul(pagg[:, 512:C], lhsT=asn[:r, :].bitcast(F32),
                                     rhs=hs[:r, 512:C], start=(c == 0), stop=(c == 4))

                ccl = sm.tile([1, KCTX], F32, tag="ccl")
                nc.vector.tensor_scalar(out=ccl[:], in0=pcnt[:], scalar1=1.0,
                                        scalar2=None, op0=OP.max)
                rcc = sm.tile([1, KCTX], F32, tag="rcc")
                nc.vector.reciprocal(out=rcc[:], in_=ccl[:])
                prc = psB.tile([KCTX, 1], F32, tag="tr", space="PSUM")
                nc.tensor.transpose(out=prc[:], in_=rcc[:], identity=ident[0:1, 0:1])
                rcol = sm.tile([KCTX, 1], F32, tag="rcol")
                nc.vector.tensor_copy(out=rcol[:], in_=prc[:])

                aggn = outc.tile([KCTX, C], F32, tag="aggn")
                nc.vector.tensor_scalar(out=aggn[:], in0=pagg[:], scalar1=rcol[:],
                                        scalar2=None, op0=OP.mult)
                ctxb = outc.tile([KCTX, C], F32, tag="ctxb")
                nc.gpsimd.indirect_dma_start(
                    out=ctxb[:], out_offset=None, in_=hidden[:, :],
                    in_offset=bass.IndirectOffsetOnAxis(ap=tgtg[:, 0:1], axis=0))
                nc.vector.tensor_add(out=aggn[:], in0=aggn[:], in1=ctxb[:])
                nc.sync.dma_start(out[b * OROW + D1:b * OROW + D1 + KCTX, :], aggn[:])

    nc.compile()
    return nc


_NC = None
_LAST_RESULTS = None


def _get_program():
    global _NC
    if _NC is None:
        _NC = build_program()
    return _NC


def _consts():
    hsel = np.zeros((2 * H, BPC), np.float32)
    for b in range(BPC):
        hsel[H * b:H * (b + 1), b] = 1.0
    w01 = np.array([[0.4], [0.6]], np.float32)
    iotaf = np.tile(np.arange(640, dtype=np.float32), (2, 1))
    iotai = np.arange(640, dtype=np.int32)
    rb = np.zeros((2, 4), np.float32)
    for b in range(BPC):
        rb[b] = [OROW * b - 1, (NF + 1) * b, L * b, 0.0]
    return dict(hsel=hsel, w01=w01, iotaf=iotaf, iotai=iotai, rb=rb)


def kernel(attn_weights, hidden_states, metric, dominant_num, contextual_num):
    from concourse.bass_utils import run_bass_kernel_spmd

    assert int(dominant_num) == DOM and int(contextual_num) == KCTX
    attn_weights = np.ascontiguousarray(attn_weights, dtype=np.float32)
    hidden_states = np.ascontiguousarray(hidden_states, dtype=np.float32)
    metric = np.ascontiguousarray(metric, dtype=np.float32)

    nc = _get_program()
    cst = _consts()
    in_maps = []
    for core in range(NCORES):
        b0 = core * BPC
        in_maps.append({
            "attn": attn_weights[b0:b0 + BPC],
            "hidden": hidden_states[b0:b0 + BPC].reshape(BPC * L, C),
            "metric": metric[b0:b0 + BPC].reshape(BPC * L, CK),
            **cst,
        })
    res = run_bass_kernel_spmd(nc, in_maps, core_ids=list(range(NCORES)))
    global _LAST_RESULTS
    _LAST_RESULTS = res
    outs = []
    idxs = []
    for core in range(NCORES):
        o = res.results[core]["out"].reshape(BPC, OROW, C)[:, :D1 + KCTX, :]
        outs.append(o)
        idxs.append(res.results[core]["all_idx"])
    return np.concatenate(outs, 0), np.concatenate(idxs, 0).astype(np.int32)
